# revision 12
# baseline (speedup 1.0000x reference)
"""Trainium2 Bass kernel for nn_Decoder (dual-attention + GRU step + vocab projection).

Sharding: data-parallel over batch B=128 (16 per core) for attention/GRU;
tensor-parallel over vocab V=30000 (3750 per core) for the output projection,
with an AllGather of [h1|ctx] and an AllGather of per-core log-softmax stats.
"""

import numpy as np

import concourse.bass as bass
import concourse.mybir as mybir
import concourse.tile as tile
from concourse.bass_utils import run_bass_kernel_spmd
from concourse.masks import make_identity

B, T, H, E = 128, 512, 512, 512
V_IN, V_OUT = 50000, 30000
NCORES = 8
BC = B // NCORES          # 16 batch rows per core
VC = V_OUT // NCORES      # 3750 vocab rows per core
VCP = 3840                # padded vocab shard (fp32r needs even/aligned N)
VCH = 384                 # logits chunk width (>=256 keeps fp32r at full rate)
NVCH = VCP // VCH         # 10 chunks
KH = H // 128             # 4 k-tiles over H
F32 = mybir.dt.float32
F32R = mybir.dt.float32r
AX = mybir.AxisListType.X
AF = mybir.ActivationFunctionType
ALU = mybir.AluOpType


def _r(ap):
    return ap.bitcast(F32R)


def split_sync_waits(nc, max_waits=1):
    """walrus setupSyncWait rejects >1-2 sync waits per CTRL instruction; move
    overflow waits onto preceding same-engine NoOps."""
    n = 0
    for bb in nc.main_func.blocks:
        new_insts = []
        for inst in bb.instructions:
            si = inst.sync_info
            if si is not None and si.on_wait and len(si.on_wait) > max_waits:
                waits = list(si.on_wait)
                extra, keep = waits[:-max_waits], waits[-max_waits:]
                for i in range(0, len(extra), max_waits):
                    chunk = extra[i : i + max_waits]
                    nop = mybir.InstNoOp(name=f"{inst.name}-wsplit{n}", ins=[], outs=[])
                    n += 1
                    nop.engine = inst.engine
                    nop.sync_info = mybir.SyncInfo(on_wait=chunk, on_update=[])
                    new_insts.append(nop)
                inst.sync_info = mybir.SyncInfo(
                    on_wait=keep, on_update=list(si.on_update)
                )
            new_insts.append(inst)
        bb.instructions[:] = new_insts


def emit_body(nc, tc, d, rep):
    """Emit one full decoder step. d: dict of dram tensor handles."""
    ts = bass.ts

    with (
        tc.tile_pool(name=f"singles{rep}", bufs=1) as singles,
        tc.tile_pool(name=f"ps_big{rep}", bufs=4, space="PSUM") as ps_big,
        tc.tile_pool(name=f"ps_s{rep}", bufs=2, space="PSUM") as ps_s,
        tc.tile_pool(name=f"ps_w{rep}", bufs=2, space="PSUM") as ps_w,
        tc.tile_pool(name=f"dram{rep}", bufs=1, space="DRAM") as dram,
    ):
        ident = singles.tile([128, 128], F32)
        make_identity(nc, ident[:])
        identR = singles.tile([128, 128], F32R)
        nc.vector.tensor_copy(out=identR[:], in_=ident[:])
        ones32 = singles.tile([1, 128], F32)
        nc.vector.memset(ones32[:], 1.0)
        ones_sb = singles.tile([1, 128], F32R)
        nc.vector.tensor_copy(out=ones_sb[:], in_=ones32[:])

        h0T_sb = singles.tile([128, KH, BC], F32R)
        nc.sync.dma_start(
            out=h0T_sb[:], in_=d["h0T"].rearrange("(n p) b -> p n b", p=128)
        )
        h0_sb = singles.tile([BC, H], F32)
        nc.sync.dma_start(out=h0_sb[:], in_=d["h0"][:])
        embT_sb = singles.tile([128, KH, BC], F32R)
        nc.sync.dma_start(
            out=embT_sb[:], in_=d["embT"].rearrange("(n p) b -> p n b", p=128)
        )

        # ctx_T accumulators (per att), [128, BC] per k-tile
        ctxT = {}
        for att in ("code", "ast"):
            ctxT[att] = [
                singles.tile(
                    [128, BC], F32, name=f"ctxT_{att}{kt}", tag=f"ctxT_{att}{kt}"
                )
                for kt in range(KH)
            ]
        ctx_nat = singles.tile([BC, H], F32)
        h1_sb = singles.tile([BC, H], F32)

        # ---------------- attention (code, then ast) ----------------
        for att in ("code", "ast"):
            with (
                tc.tile_pool(name=f"att_w{att}{rep}", bufs=1) as attw,
                tc.tile_pool(name=f"att_enc{att}{rep}", bufs=4) as encp,
                tc.tile_pool(name=f"att_relu{att}{rep}", bufs=2) as relup,
                tc.tile_pool(name=f"att_misc{att}{rep}", bufs=2) as miscp,
            ):
                # WT: [3H, H] = W_attn.T; rows [0:H)=W1T, [H:2H)=W2T, [2H:3H)=W3T
                WT_sb = attw.tile([128, 3 * KH, H], F32R)
                nc.sync.dma_start(
                    out=WT_sb[:],
                    in_=d[f"WT_{att}"].rearrange("(n p) h -> p n h", p=128),
                )
                v_sb = attw.tile([128, KH], F32R)
                nc.sync.dma_start(out=v_sb[:], in_=d[f"v_{att}"][:])
                wc_sb = attw.tile([128, KH], F32R)
                nc.sync.dma_start(out=wc_sb[:], in_=d[f"wc_{att}"][:])
                battn_sb = attw.tile([128, KH], F32)
                nc.sync.dma_start(out=battn_sb[:], in_=d[f"battn_{att}"][:])
                cov_sb = attw.tile([BC, T], F32)
                nc.sync.dma_start(out=cov_sb[:], in_=d[f"cov_{att}"][:])
                # flat single-partition copies: matmul rhs must start at
                # partition 0, so per-b rows live at [0:1, b*T:(b+1)*T]
                cov_flat = attw.tile([1, BC * T], F32R)
                nc.sync.dma_start(out=cov_flat[:], in_=d[f"covflat_{att}"][:])
                w_flat = attw.tile([1, BC * T], F32R)
                w_nat = attw.tile([BC, T], F32)

                # u_row[1, H] = Wc @ W3T   (u = W3 @ Wc)
                u_ps = ps_s.tile([1, H], F32, tag="sps")
                for kt in range(KH):
                    nc.tensor.matmul(
                        u_ps[:],
                        lhsT=(wc_sb[:, kt : kt + 1]),
                        rhs=(WT_sb[:, 2 * KH + kt, :]),
                        start=(kt == 0),
                        stop=(kt == KH - 1),
                    )
                u_sb = attw.tile([1, H], F32R)
                nc.vector.tensor_copy(out=u_sb[:], in_=u_ps[:])

                # c_T[hout, b] = W1 @ h0.T + b_attn  -> c_sb tiles [128, BC]
                c_sb = []
                for ht in range(KH):
                    c_ps = ps_big.tile([128, BC], F32, tag="eps")
                    for kt in range(KH):
                        nc.tensor.matmul(
                            c_ps[:],
                            lhsT=(WT_sb[:, kt, ts(ht, 128)]),
                            rhs=(h0T_sb[:, kt, :]),
                            start=(kt == 0),
                            stop=(kt == KH - 1),
                        )
                    ct = attw.tile([128, BC], F32, tag=f"c{ht}")
                    nc.scalar.activation(
                        out=ct[:],
                        in_=c_ps[:],
                        func=AF.Identity,
                        bias=battn_sb[:, ht : ht + 1],
                        scale=1.0,
                    )
                    c_sb.append(ct)

                encT_d = d[f"encT_{att}"]  # [H, BC, T]
                for b in range(BC):
                    et = []
                    for kt in range(KH):
                        e_t = encp.tile([128, T], F32R, tag=f"enc{kt}")
                        nc.sync.dma_start(
                            out=e_t[:], in_=encT_d[ts(kt, 128), b, :]
                        )
                        et.append(e_t)

                    # energy_T[hout, t] (per hout tile) + relu(+c) + v-dot
                    s_ps = ps_s.tile([1, T], F32, tag="sps")
                    relu_t = []
                    for ht in range(KH):
                        e_ps = ps_big.tile([128, T], F32, tag="eps")
                        for kt in range(KH):
                            nc.tensor.matmul(
                                e_ps[:],
                                lhsT=(WT_sb[:, KH + kt, ts(ht, 128)]),
                                rhs=(et[kt][:]),
                                start=(kt == 0),
                                stop=False,
                            )
                        # + u[hout] * cov[b, t]  (K=1)
                        nc.tensor.matmul(
                            e_ps[:],
                            lhsT=(u_sb[0:1, ts(ht, 128)]),
                            rhs=(cov_flat[0:1, bass.ts(b, T)]),
                            start=False,
                            stop=True,
                        )
                        rl = relup.tile([128, T], F32R, tag=f"relu{ht}")
                        nc.scalar.activation(
                            out=rl[:],
                            in_=e_ps[:],
                            func=AF.Relu,
                            bias=c_sb[ht][:, b : b + 1],
                            scale=1.0,
                        )
                        relu_t.append(rl)
                    for ht in range(KH):
                        nc.tensor.matmul(
                            s_ps[:],
                            lhsT=(v_sb[:, ht : ht + 1]),
                            rhs=(relu_t[ht][:]),
                            start=(ht == 0),
                            stop=(ht == KH - 1),
                        )

                    # per-b softmax over t (row [1, T])
                    negm = miscp.tile([1, 1], F32, tag="negm")
                    nc.vector.reduce_max(out=negm[:], in_=s_ps[:], axis=AX, negate=True)
                    p_row = miscp.tile([1, T], F32, tag="prow")
                    nc.scalar.activation(
                        out=p_row[:],
                        in_=s_ps[:],
                        func=AF.Exp,
                        bias=negm[0:1, 0:1],
                        scale=1.0,
                    )
                    ssum = miscp.tile([1, 1], F32, tag="ssum")
                    nc.vector.reduce_sum(out=ssum[:], in_=p_row[:], axis=AX)
                    rsum = miscp.tile([1, 1], F32, tag="rsum")
                    nc.vector.reciprocal(out=rsum[:], in_=ssum[:])
                    nc.vector.tensor_scalar_mul(
                        out=w_flat[0:1, bass.ts(b, T)],
                        in0=p_row[:],
                        scalar1=rsum[0:1, 0:1],
                    )

                    # broadcast w row across partitions (K=1 matmul with ones)
                    wb_ps = ps_w.tile([128, T], F32, tag="wps")
                    nc.tensor.matmul(
                        wb_ps[:],
                        lhsT=(ones_sb[:]),
                        rhs=(w_flat[0:1, bass.ts(b, T)]),
                        start=True,
                        stop=True,
                    )
                    # ctx_T[:, b] += sum_t encT * w  (fused mul+reduce on DVE)
                    for kt in range(KH):
                        scr = miscp.tile([128, T], F32, tag="ctxscr")
                        nc.vector.tensor_mul(
                            out=scr[:], in0=et[kt][:].bitcast(F32), in1=wb_ps[:]
                        )
                        nc.vector.reduce_sum(
                            out=ctxT[att][kt][:, b : b + 1], in_=scr[:], axis=AX
                        )

                # outputs for this attention: w and cov+w
                nc.sync.dma_start(
                    out=d[f"w_{att}_out"].rearrange("b t -> (b t)")[None],
                    in_=w_flat[:].bitcast(F32),
                )
                # w back to [BC, T] layout for the coverage update: bounce
                # through the already-written DRAM output (partition-crossing
                # SBUF views are not addressable directly)
                nc.sync.dma_start(out=w_nat[:], in_=d[f"w_{att}_out"][:])
                ccov = attw.tile([BC, T], F32, tag="ccov")
                nc.vector.tensor_add(out=ccov[:], in0=cov_sb[:], in1=w_nat[:])
                nc.sync.dma_start(out=d[f"ccov_{att}_out"][:], in_=ccov[:])

        # ---------------- combine ctx, GRU ----------------
        with (
            tc.tile_pool(name=f"gru{rep}", bufs=1) as grup,
            tc.tile_pool(name=f"gru_s{rep}", bufs=2) as grus,
        ):
            # ctx = 0.5*(code + ast): keep transposed tiles for matmul lhsT,
            # also build natural [BC, H] for the xh allgather
            ctxTc = []
            for kt in range(KH):
                cc = grup.tile([128, BC], F32R, tag=f"ctxc{kt}")
                nc.vector.tensor_add(
                    out=cc[:], in0=ctxT["code"][kt][:], in1=ctxT["ast"][kt][:]
                )
                nc.vector.tensor_scalar_mul(out=cc[:], in0=cc[:], scalar1=0.5)
                ctxTc.append(cc)
                ct_ps = ps_w.tile([BC, 128], F32R, tag="wps")
                nc.tensor.transpose(ct_ps[:], cc[:], identR[:])
                nc.vector.tensor_copy(
                    out=ctx_nat[:, ts(kt, 128)], in_=ct_ps[:].bitcast(F32)
                )

            W_ihT_sb = grup.tile([128, 8, 3 * H], F32R)
            nc.sync.dma_start(
                out=W_ihT_sb[:], in_=d["W_ihT"].rearrange("(n p) h -> p n h", p=128)
            )
            W_hhT_sb = grup.tile([128, KH, 3 * H], F32R)
            nc.sync.dma_start(
                out=W_hhT_sb[:], in_=d["W_hhT"].rearrange("(n p) h -> p n h", p=128)
            )
            brz_sb = grup.tile([1, 2 * H], F32R)
            nc.sync.dma_start(out=brz_sb[:], in_=d["bsum_rz"][:])
            bihn_sb = grup.tile([1, H], F32R)
            nc.sync.dma_start(out=bihn_sb[:], in_=d["bihn"][:])
            bhhn_sb = grup.tile([1, H], F32R)
            nc.sync.dma_start(out=bhhn_sb[:], in_=d["bhhn"][:])

            def xT(kt):  # x = [emb, ctx], k-tile kt of x^T
                return embT_sb[:, kt, :] if kt < KH else ctxTc[kt - KH][:]

            # r, z gates: gi + gh + (b_ih + b_hh)
            gates = []
            for ch in range(2):
                g_ps = ps_big.tile([BC, H], F32, tag="eps")
                for kt in range(8):
                    nc.tensor.matmul(
                        g_ps[:],
                        lhsT=(xT(kt)),
                        rhs=(W_ihT_sb[:, kt, ts(ch, H)]),
                        start=(kt == 0),
                        stop=False,
                    )
                for kt in range(KH):
                    nc.tensor.matmul(
                        g_ps[:],
                        lhsT=(h0T_sb[:, kt, :]),
                        rhs=(W_hhT_sb[:, kt, ts(ch, H)]),
                        start=False,
                        stop=False,
                    )
                nc.tensor.matmul(
                    g_ps[:],
                    lhsT=(ones_sb[0:1, :BC]),
                    rhs=(brz_sb[0:1, ts(ch, H)]),
                    start=False,
                    stop=True,
                )
                g_sb = grus.tile([BC, H], F32, tag=f"gate{ch}")
                nc.scalar.activation(out=g_sb[:], in_=g_ps[:], func=AF.Sigmoid)
                gates.append(g_sb)
            r_sb, z_sb = gates

            # i_n and h_n kept separate (n = tanh(i_n + r*h_n))
            in_ps = ps_big.tile([BC, H], F32, tag="eps")
            for kt in range(8):
                nc.tensor.matmul(
                    in_ps[:],
                    lhsT=(xT(kt)),
                    rhs=(W_ihT_sb[:, kt, ts(2, H)]),
                    start=(kt == 0),
                    stop=False,
                )
            nc.tensor.matmul(
                in_ps[:],
                lhsT=(ones_sb[0:1, :BC]),
                rhs=(bihn_sb[:]),
                start=False,
                stop=True,
            )
            hn_ps = ps_big.tile([BC, H], F32, tag="eps")
            for kt in range(KH):
                nc.tensor.matmul(
                    hn_ps[:],
                    lhsT=(h0T_sb[:, kt, :]),
                    rhs=(W_hhT_sb[:, kt, ts(2, H)]),
                    start=(kt == 0),
                    stop=False,
                )
            nc.tensor.matmul(
                hn_ps[:],
                lhsT=(ones_sb[0:1, :BC]),
                rhs=(bhhn_sb[:]),
                start=False,
                stop=True,
            )
            rhn = grus.tile([BC, H], F32, tag="rhn")
            nc.vector.tensor_mul(out=rhn[:], in0=r_sb[:], in1=hn_ps[:])
            npre = grus.tile([BC, H], F32, tag="npre")
            nc.vector.tensor_add(out=npre[:], in0=rhn[:], in1=in_ps[:])
            n_sb = grus.tile([BC, H], F32, tag="nsb")
            nc.scalar.activation(out=n_sb[:], in_=npre[:], func=AF.Tanh)
            # h1 = n + z*(h0 - n)
            dsb = grus.tile([BC, H], F32, tag="dsb")
            nc.vector.tensor_sub(out=dsb[:], in0=h0_sb[:], in1=n_sb[:])
            nc.vector.tensor_mul(out=dsb[:], in0=z_sb[:], in1=dsb[:])
            nc.vector.tensor_add(out=h1_sb[:], in0=n_sb[:], in1=dsb[:])
            nc.sync.dma_start(out=d["h1_out"][:], in_=h1_sb[:])

        # ---------------- allgather xh = [h1 | ctx] ----------------
        xh_dram = dram.tile([BC, 2 * H], F32)
        xhg_dram = dram.tile([B, 2 * H], F32)
        nc.sync.dma_start(out=xh_dram[:, :H], in_=h1_sb[:])
        nc.sync.dma_start(out=xh_dram[:, H:], in_=ctx_nat[:])
        nc.gpsimd.collective_compute(
            "AllGather",
            ALU.bypass,
            ins=[xh_dram.opt()],
            outs=[xhg_dram.opt()],
            replica_groups=[list(range(NCORES))],
        )

        # ---------------- phase 2: logits chunk + log_softmax ----------------
        with (
            tc.tile_pool(name=f"ph2{rep}", bufs=1) as ph2,
            tc.tile_pool(name=f"wo{rep}", bufs=3) as wop,
        ):
            xh_sb = ph2.tile([B, 2 * H], F32)
            nc.sync.dma_start(out=xh_sb[:], in_=xhg_dram[:])
            xhT = []
            for kt in range(8):
                xp = ps_w.tile([128, 128], F32, tag="wps")
                nc.tensor.transpose(xp[:], xh_sb[:, ts(kt, 128)], ident[:])
                xt_sb = ph2.tile([128, 128], F32R, tag=f"xhT{kt}")
                nc.vector.tensor_copy(out=xt_sb[:], in_=xp[:])
                xhT.append(xt_sb)

            bo_sb = ph2.tile([1, VCP], F32R)
            nc.sync.dma_start(out=bo_sb[:], in_=d["bo"][:])
            logits_sb = ph2.tile([B, VCP], F32)

            wo_sb = []
            for kt in range(8):
                wt = wop.tile([128, VCP], F32R, tag=f"wo{kt % 3}")
                nc.sync.dma_start(out=wt[:], in_=d["WoT"][ts(kt, 128), :])
                wo_sb.append(wt)

            for ch in range(NVCH):
                sl = ts(ch, VCH)
                l_ps = ps_big.tile([B, VCH], F32, tag="eps")
                for kt in range(8):
                    nc.tensor.matmul(
                        l_ps[:],
                        lhsT=(xhT[kt][:]),
                        rhs=(wo_sb[kt][:, sl]),
                        start=(kt == 0),
                        stop=False,
                    )
                nc.tensor.matmul(
                    l_ps[:],
                    lhsT=(ones_sb[0:1, :B]),
                    rhs=(bo_sb[0:1, sl]),
                    start=False,
                    stop=True,
                )
                nc.scalar.activation(
                    out=logits_sb[:, sl], in_=l_ps[:], func=AF.Identity
                )

            negmax = ph2.tile([B, 1], F32)
            nc.vector.reduce_max(out=negmax[:], in_=logits_sb[:], axis=AX, negate=True)
            escr = ph2.tile([B, VCP], F32)
            locsum = ph2.tile([B, 1], F32)
            nc.scalar.activation(
                out=escr[:],
                in_=logits_sb[:],
                func=AF.Exp,
                bias=negmax[:, 0:1],
                scale=1.0,
                accum_out=locsum[:],
            )
            stats_sb = ph2.tile([B, 2], F32)
            nc.vector.tensor_scalar_mul(
                out=stats_sb[:, 0:1], in0=negmax[:], scalar1=-1.0
            )
            nc.vector.tensor_copy(out=stats_sb[:, 1:2], in_=locsum[:])
            stats_dram = dram.tile([B, 2], F32)
            gstats_dram = dram.tile([NCORES * B, 2], F32)
            nc.sync.dma_start(out=stats_dram[:], in_=stats_sb[:])
            nc.gpsimd.collective_compute(
                "AllGather",
                ALU.bypass,
                ins=[stats_dram.opt()],
                outs=[gstats_dram.opt()],
                replica_groups=[list(range(NCORES))],
            )
            gst_sb = ph2.tile([B, NCORES, 2], F32)
            nc.sync.dma_start(
                out=gst_sb[:], in_=gstats_dram.rearrange("(c b) s -> b c s", b=B)
            )
            gmax_neg = ph2.tile([B, 1], F32)
            nc.vector.reduce_max(
                out=gmax_neg[:], in_=gst_sb[:, :, 0], axis=AX, negate=True
            )
            e8 = ph2.tile([B, NCORES], F32)
            nc.scalar.activation(
                out=e8[:],
                in_=gst_sb[:, :, 0],
                func=AF.Exp,
                bias=gmax_neg[:, 0:1],
                scale=1.0,
            )
            sc8 = ph2.tile([B, NCORES], F32)
            nc.vector.tensor_mul(out=sc8[:], in0=e8[:], in1=gst_sb[:, :, 1])
            S_sb = ph2.tile([B, 1], F32)
            nc.vector.reduce_sum(out=S_sb[:], in_=sc8[:], axis=AX)
            lnS = ph2.tile([B, 1], F32)
            nc.scalar.activation(out=lnS[:], in_=S_sb[:], func=AF.Ln)
            lse = ph2.tile([B, 1], F32)
            # lse = gmax + ln(S) = lnS - gmax_neg
            nc.vector.tensor_sub(out=lse[:], in0=lnS[:], in1=gmax_neg[:])
            nc.vector.tensor_scalar_sub(
                out=escr[:], in0=logits_sb[:], scalar1=lse[:, 0:1]
            )
            nc.sync.dma_start(out=d["out_chunk"][:], in_=escr[:])


def build_kernel(reps=1):
    nc = bass.Bass("TRN2", num_devices=NCORES)
    d = {}

    def din(name, shape, dt=F32):
        d[name] = nc.dram_tensor(name, shape, dt, kind="ExternalInput")

    def dout(name, shape):
        d[name] = nc.dram_tensor(name, shape, F32, kind="ExternalOutput")

    for att in ("code", "ast"):
        din(f"encT_{att}", [H, BC, T], F32R)
        din(f"cov_{att}", [BC, T])
        din(f"covflat_{att}", [1, BC * T], F32R)
        din(f"WT_{att}", [3 * H, H], F32R)
        din(f"v_{att}", [128, KH], F32R)
        din(f"wc_{att}", [128, KH], F32R)
        din(f"battn_{att}", [128, KH])
    din("h0", [BC, H])
    din("h0T", [H, BC], F32R)
    din("embT", [E, BC], F32R)
    din("W_ihT", [E + H, 3 * H], F32R)
    din("W_hhT", [H, 3 * H], F32R)
    din("bsum_rz", [1, 2 * H], F32R)
    din("bihn", [1, H], F32R)
    din("bhhn", [1, H], F32R)
    din("WoT", [2 * H, VCP], F32R)
    din("bo", [1, VCP], F32R)

    dout("out_chunk", [B, VCP])
    dout("h1_out", [BC, H])
    for att in ("code", "ast"):
        dout(f"w_{att}_out", [BC, T])
        dout(f"ccov_{att}_out", [BC, T])

    with tile.TileContext(nc) as tc:
        for rep in range(reps):
            emit_body(nc, tc, d, rep)
    split_sync_waits(nc)
    return nc


def _prep_in_maps(inputs):
    f32 = lambda a: np.ascontiguousarray(np.asarray(a), dtype=np.float32)
    code_outputs = f32(inputs["code_outputs"])  # [T, B, H]
    ast_outputs = f32(inputs["ast_outputs"])
    code_cov = f32(inputs["code_coverage"])  # [B, T]
    ast_cov = f32(inputs["ast_coverage"])
    h0 = f32(inputs["last_hidden"])[0]  # [B, H]
    emb = f32(inputs["embedding"])[np.asarray(inputs["inputs"])]  # [B, E]

    shared = {}
    for att, W, bb, v, wc in (
        ("code", "code_W_attn", "code_b_attn", "code_v", "code_W_cov"),
        ("ast", "ast_W_attn", "ast_b_attn", "ast_v", "ast_W_cov"),
    ):
        shared[f"WT_{att}"] = f32(np.asarray(inputs[W]).T)  # [3H, H]
        shared[f"battn_{att}"] = f32(np.asarray(inputs[bb]).reshape(KH, 128).T)
        shared[f"v_{att}"] = f32(np.asarray(inputs[v]).reshape(KH, 128).T)
        shared[f"wc_{att}"] = f32(np.asarray(inputs[wc]).reshape(KH, 128).T)
    shared["W_ihT"] = f32(np.asarray(inputs["gru_W_ih"]).T)  # [E+H, 3H]
    shared["W_hhT"] = f32(np.asarray(inputs["gru_W_hh"]).T)  # [H, 3H]
    b_ih = f32(inputs["gru_b_ih"])
    b_hh = f32(inputs["gru_b_hh"])
    shared["bsum_rz"] = (b_ih + b_hh)[None, : 2 * H]
    shared["bihn"] = b_ih[None, 2 * H :]
    shared["bhhn"] = b_hh[None, 2 * H :]
    W_out = f32(inputs["W_out"])  # [V, 2H]
    b_out = f32(inputs["b_out"])

    in_maps = []
    for c in range(NCORES):
        bsl = slice(c * BC, (c + 1) * BC)
        vsl = slice(c * VC, (c + 1) * VC)
        m = dict(shared)
        m["encT_code"] = f32(code_outputs[:, bsl, :].transpose(2, 1, 0))
        m["encT_ast"] = f32(ast_outputs[:, bsl, :].transpose(2, 1, 0))
        m["cov_code"] = code_cov[bsl]
        m["cov_ast"] = ast_cov[bsl]
        m["covflat_code"] = code_cov[bsl].reshape(1, -1)
        m["covflat_ast"] = ast_cov[bsl].reshape(1, -1)
        m["h0"] = h0[bsl]
        m["h0T"] = f32(h0[bsl].T)
        m["embT"] = f32(emb[bsl].T)
        woT = np.zeros((2 * H, VCP), np.float32)
        woT[:, :VC] = W_out[vsl].T
        m["WoT"] = woT
        bo = np.full((1, VCP), -1e30, np.float32)  # pad cols vanish in softmax
        bo[0, :VC] = b_out[vsl]
        m["bo"] = bo
        in_maps.append(m)
    return in_maps


_NC_CACHE = {}


def kernel(reps=1, **inputs):
    if reps not in _NC_CACHE:
        _NC_CACHE[reps] = build_kernel(reps)
    nc = _NC_CACHE[reps]
    in_maps = _prep_in_maps(inputs)
    res = run_bass_kernel_spmd(nc, in_maps, core_ids=list(range(NCORES)))
    rs = res.results
    out = np.concatenate(
        [rs[c]["out_chunk"][:, :VC] for c in range(NCORES)], axis=1
    )
    h1 = np.concatenate([rs[c]["h1_out"] for c in range(NCORES)], axis=0)[None]
    cw = np.concatenate([rs[c]["w_code_out"] for c in range(NCORES)], axis=0)[:, None, :]
    aw = np.concatenate([rs[c]["w_ast_out"] for c in range(NCORES)], axis=0)[:, None, :]
    ccov = np.concatenate([rs[c]["ccov_code_out"] for c in range(NCORES)], axis=0)
    acov = np.concatenate([rs[c]["ccov_ast_out"] for c in range(NCORES)], axis=0)
    return (out, h1, cw, aw, ccov, acov)


if __name__ == "__main__":
    nc = build_kernel(reps=1)
    n_inst = sum(len(bb.instructions) for bb in nc.main_func.blocks)
    print(f"built OK: {n_inst} instructions")
